# revision 1
# baseline (speedup 1.0000x reference)
"""Trainium2 Bass kernel for nn_BatchDifferentiableKF.

Problem: batched 4-state Kalman filter, B=16384 batch rows, T=512 steps,
state [px, py, vx, vy], measurements = predicted velocities (B, T, 2).

Key structure exploited:
  * The covariance/gain recursion is data-independent (P0 = I for every
    batch row), so the Kalman gains k_p[t], k_v[t] are a fixed schedule
    computed on host in float64.
  * x/y components decouple into two identical scalar filters:
        p_t = p_{t-1} + g[t] v_{t-1} + k_p[t] z_t      (g = dt - k_p)
        v_t = a[t] v_{t-1} + k_v[t] z_t                (a = 1 - k_v)
    i.e. the whole filter is LINEAR in (z, p0).
  * Chunk T into 4 x 128. Within a chunk the map (carry, z) -> outputs is a
    dense lower-triangular 128x128 matrix pair (Wp, Wv) plus rank-2 carry
    terms; chunks 1..3 share identical (steady-state) weights.

Device algorithm per core (2048 batch rows):
  1. Load z fp32 batch-tiles, cast to bf16 (DVE/ACT), transpose on the
     TensorEngine into time-major tiles ZT[k] [128 (s,c) x 2048 b] bf16.
  2. Per (chunk m, batch-tile j): 3 PE matmuls produce out[128 b, 512
     (pv,t,c)] in PSUM (2x bf16 data x weights + bf16 carry x coeffs);
     batched boundary matmuls advance the bf16 carry; DVE/ACT evacuate
     PSUM; HWDGE/SWDGE DMAs write pos/vel chunks (contiguous 1KB rows).

Sharding: embarrassingly parallel over batch across the 8 cores.
"""

import numpy as np
import ml_dtypes

B_FULL = 16384
T = 512
C = 128          # chunk length
NCH = T // C     # 4 chunks
N_CORES = 8
B_CORE = B_FULL // N_CORES   # 2048
VEL_KEEP = 32    # vel carry columns kept (a_ss^32 ~ 5e-19, below fp32)
CW_N = 2 * C + 2 * VEL_KEEP  # 320 carry-matmul columns


# ----------------------------------------------------------------------------
# Host-side weight construction (float64)
# ----------------------------------------------------------------------------

def _gains(dt, q_pos, q_vel, r_vel, n):
    """Gain schedule k_p[t], k_v[t] of the decoupled scalar filter, P0=I."""
    dt = float(np.float32(dt))
    r = float(np.float32(r_vel)) + float(np.float32(1e-6))
    qp = float(np.float32(q_pos))
    qv = float(np.float32(q_vel))
    Ppp, Ppv, Pvv = 1.0, 0.0, 1.0
    k_p = np.zeros(n)
    k_v = np.zeros(n)
    for t in range(n):
        Ppv_ = Ppv + dt * Pvv
        Ppp_ = Ppp + 2.0 * dt * Ppv + dt * dt * Pvv + qp
        Pvv_ = Pvv + qv
        S = Pvv_ + r
        k_p[t] = Ppv_ / S
        k_v[t] = Pvv_ / S
        Ppp = Ppp_ - k_p[t] * Ppv_
        Ppv = Ppv_ - k_p[t] * Pvv_
        Pvv = Pvv_ - k_v[t] * Pvv_
    return k_p, k_v


def _chunk_maps(k_p, k_v, dt):
    """Per-chunk affine maps: (p_in, v_in, z[0..C-1]) -> (p[0..C-1], v[..]).

    p_out[i] = p_in + Bv[m][i] v_in + sum_j Wp[m][i,j] z[j]
    v_out[i] =        Av[m][i] v_in + sum_j Wv[m][i,j] z[j]
    """
    g = dt - k_p
    a = 1.0 - k_v
    Wp = np.zeros((NCH, C, C))
    Wv = np.zeros((NCH, C, C))
    Av = np.zeros((NCH, C))
    Bv = np.zeros((NCH, C))
    for m in range(NCH):
        pcoef = np.zeros(C + 1)
        vcoef = np.zeros(C + 1)
        vcoef[0] = 1.0
        for i in range(C):
            t = m * C + i
            pcoef = pcoef + g[t] * vcoef
            pcoef[1 + i] += k_p[t]
            vcoef = a[t] * vcoef
            vcoef[1 + i] += k_v[t]
            Bv[m, i] = pcoef[0]
            Wp[m, i] = pcoef[1:]
            Av[m, i] = vcoef[0]
            Wv[m, i] = vcoef[1:]
    return Wp, Wv, Av, Bv


def build_weights(dt, q_pos, q_vel, r_vel):
    """Device constant tensors. Layouts:

    partition index q of a transposed-data tile <-> (j_local = 64h + q//2,
    c = q&1) for tile half h; output free index f = pv*256 + t*2 + c'.
    Carry stream order e: 0,1 = p_in (c'=0,1); 2,3 = v_in (c'=0,1).
    """
    dtf = float(np.float32(dt))
    k_p, k_v = _gains(dt, q_pos, q_vel, r_vel, T)
    Wp, Wv, Av, Bv = _chunk_maps(k_p, k_v, dtf)

    bf16 = ml_dtypes.bfloat16
    out = {}
    for mset in range(2):
        mc = mset  # chunk-map index (chunk 0, or steady chunk 1)
        for h in range(2):
            w = np.zeros((128, 512))
            for q in range(128):
                j = 64 * h + q // 2
                c = q & 1
                # f = pv*256 + t*2 + c ; delta_{c,c'} keeps only c'==c
                w[q, 0 * 256 + 2 * np.arange(C) + c] = Wp[mc, :, j]
                w[q, 1 * 256 + 2 * np.arange(C) + c] = Wv[mc, :, j]
            out[f"wmain_{mset}_{h}"] = w.astype(bf16)
            bw = np.zeros((128, 4))
            for q in range(128):
                j = 64 * h + q // 2
                c = q & 1
                bw[q, c] = Wp[mc, C - 1, j]
                bw[q, 2 + c] = Wv[mc, C - 1, j]
            out[f"bw_{mset}_{h}"] = bw.astype(bf16)
        cw = np.zeros((4, CW_N))
        for cp in range(2):
            cw[cp, 2 * np.arange(C) + cp] = 1.0
            cw[2 + cp, 2 * np.arange(C) + cp] = Bv[mc]
            cw[2 + cp, 2 * C + 2 * np.arange(VEL_KEEP) + cp] = Av[mc, :VEL_KEEP]
        out[f"cw_{mset}"] = cw.astype(bf16)
        mw = np.zeros((4, 4))
        for cp in range(2):
            mw[cp, cp] = 1.0             # p_out_c += p_in_c
            mw[2 + cp, cp] = Bv[mc, C - 1]   # p_out_c += Bend * v_in_c
            mw[2 + cp, 2 + cp] = Av[mc, C - 1]
        out[f"mw_{mset}"] = mw.astype(bf16)
    out["ident"] = np.eye(128, dtype=np.float32)
    out["identb"] = np.eye(128, dtype=bf16)
    return out


# ----------------------------------------------------------------------------
# Bass kernel
# ----------------------------------------------------------------------------

def build_nc(n_bt):
    """Build the Bass program for one core processing n_bt*128 batch rows."""
    import concourse.bass as bass
    import concourse.tile as tile
    from concourse import bacc, mybir
    from contextlib import ExitStack

    f32 = mybir.dt.float32
    f32r = mybir.dt.float32r
    bf16 = mybir.dt.bfloat16

    b_sz = n_bt * 128
    nc = bacc.Bacc("TRN2", target_bir_lowering=False, debug=False)

    z_in = nc.dram_tensor("z_in", [b_sz, 1024], f32, kind="ExternalInput").ap()
    p0_in = nc.dram_tensor("p0_in", [b_sz, 2], f32, kind="ExternalInput").ap()
    wmain_d = [[nc.dram_tensor(f"wmain_{ms}_{h}", [128, 512], bf16,
                               kind="ExternalInput").ap()
                for h in range(2)] for ms in range(2)]
    bw_d = [[nc.dram_tensor(f"bw_{ms}_{h}", [128, 4], bf16,
                            kind="ExternalInput").ap()
             for h in range(2)] for ms in range(2)]
    cw_d = [nc.dram_tensor(f"cw_{ms}", [4, CW_N], bf16,
                           kind="ExternalInput").ap() for ms in range(2)]
    mw_d = [nc.dram_tensor(f"mw_{ms}", [4, 4], bf16,
                           kind="ExternalInput").ap() for ms in range(2)]
    ident_d = nc.dram_tensor("ident", [128, 128], f32, kind="ExternalInput").ap()
    identb_d = nc.dram_tensor("identb", [128, 128], bf16,
                              kind="ExternalInput").ap()
    pos_out = nc.dram_tensor("pos_out", [b_sz, 1024], f32,
                             kind="ExternalOutput").ap()
    vel_out = nc.dram_tensor("vel_out", [b_sz, 1024], f32,
                             kind="ExternalOutput").ap()

    JG = min(4, n_bt)           # batch-tiles per carry group
    kw = 128 * JG               # carry tile width
    n_jg = n_bt // JG

    with tile.TileContext(nc) as tc, ExitStack() as ctx:
        const = ctx.enter_context(tc.tile_pool(name="const", bufs=1))
        ztp = ctx.enter_context(tc.tile_pool(name="ztp", bufs=1))
        kpool = ctx.enter_context(tc.tile_pool(name="kpool", bufs=1))
        p0p = ctx.enter_context(tc.tile_pool(name="p0p", bufs=4))
        stage = ctx.enter_context(tc.tile_pool(name="stage", bufs=2))
        ps_main = ctx.enter_context(tc.tile_pool(name="ps_main", bufs=4,
                                                 space="PSUM"))
        ps_c = ctx.enter_context(tc.tile_pool(name="ps_c", bufs=1,
                                              space="PSUM"))
        ps_p0 = ctx.enter_context(tc.tile_pool(name="ps_p0", bufs=1,
                                               space="PSUM"))

        # ---- constants -> SBUF ----
        wmain_sb = [[const.tile([128, 512], bf16, name=f"wm_{ms}_{h}",
                                tag=f"wm{ms}{h}")
                     for h in range(2)] for ms in range(2)]
        bw_sb = [[const.tile([128, 4], bf16, name=f"bwsb_{ms}_{h}",
                             tag=f"bw{ms}{h}")
                  for h in range(2)] for ms in range(2)]
        cw_sb = [const.tile([4, CW_N], bf16, name=f"cwsb_{ms}", tag=f"cw{ms}")
                 for ms in range(2)]
        mw_sb = [const.tile([4, 4], bf16, name=f"mwsb_{ms}", tag=f"mw{ms}")
                 for ms in range(2)]
        ident_sb = const.tile([128, 128], f32, name="ident_sb", tag="ident")
        identb_sb = const.tile([128, 128], bf16, name="identb_sb", tag="identb")
        for ms in range(2):
            for h in range(2):
                nc.scalar.dma_start(wmain_sb[ms][h][:], wmain_d[ms][h])
                nc.scalar.dma_start(bw_sb[ms][h][:], bw_d[ms][h])
            nc.scalar.dma_start(cw_sb[ms][:], cw_d[ms])
            nc.scalar.dma_start(mw_sb[ms][:], mw_d[ms])
        nc.scalar.dma_start(ident_sb[:], ident_d)
        nc.scalar.dma_start(identb_sb[:], identb_d)

        # ---- PE warm-up: dummy matmuls while input DMAs are in flight,
        # so the HAM clock gate reaches 2.4 GHz before real work ----
        warm_ps = ps_p0.tile([128, 128], f32, name="warm_ps", tag="pp")
        for wi in range(40):
            nc.tensor.matmul(warm_ps[:], identb_sb[:], identb_sb[:],
                             start=(wi == 0), stop=(wi == 39))

        # ZT[k] partition q <-> (s = 64k + q//2, c = q&1), free = batch.
        zt = [ztp.tile([128, b_sz], bf16, name=f"zt_{k}", tag=f"zt{k}")
              for k in range(8)]

        zfp = ctx.enter_context(tc.tile_pool(name="zfp", bufs=4))
        zbp = ctx.enter_context(tc.tile_pool(name="zbp", bufs=8))
        ps_tr = ctx.enter_context(tc.tile_pool(name="ps_tr", bufs=2,
                                               space="PSUM"))

        def load_and_cast(j):
            """z batch-tile j: DMA fp32 in, cast to bf16."""
            bsl = slice(128 * j, 128 * (j + 1))
            zf = zfp.tile([128, 1024], f32, name=f"zf_{j}", tag="zf")
            nc.sync.dma_start(zf[:], z_in[bsl, :])
            zb = zbp.tile([128, 1024], bf16, name=f"zb_{j}", tag="zb")
            nc.vector.tensor_copy(zb[:], zf[:])
            return zb

        def emit_transpose(zb, j, k):
            bsl = slice(128 * j, 128 * (j + 1))
            tp = ps_tr.tile([128, 128], bf16, name=f"tp_{j}_{k}", tag="tp")
            nc.tensor.transpose(tp[:], zb[:, 128 * k:128 * (k + 1)],
                                identb_sb[:])
            if (j + k) % 3 != 0:
                nc.vector.tensor_copy(zt[k][:, bsl], tp[:])
            else:
                nc.scalar.copy(zt[k][:, bsl], tp[:])

        # ---- phase D: initial carries from p0 (PE transpose) ----
        # K[m][jg] [4, kw] f32r: rows (p_c0, p_c1, v_c0, v_c1)
        kt = [[kpool.tile([4, kw], bf16, name=f"kt_{m}_{jg}", tag=f"k{m}_{jg}")
               for jg in range(n_jg)] for m in range(NCH)]
        for jg in range(n_jg):
            pp = ps_p0.tile([4, kw], f32)
            for jj in range(JG):
                j = jg * JG + jj
                p0_sb = p0p.tile([128, 4], f32)
                nc.vector.memset(p0_sb[:, 2:4], 0.0)
                nc.scalar.dma_start(p0_sb[:, 0:2],
                                    p0_in[128 * j:128 * (j + 1), :])
                nc.tensor.transpose(pp[:, 128 * jj:128 * (jj + 1)],
                                    p0_sb[:], ident_sb[:, :])
            nc.scalar.copy(kt[0][jg][:], pp[:])

        # ---- phase E: main loop, jg-outer. The first group's transposes
        # run upfront; each later group's transposes are sprinkled two-per-
        # matmul-group into the PE stream so the PE never idles long enough
        # for the HAM clock gate to re-throttle. ----
        zb_cur = [load_and_cast(jj) for jj in range(JG)]
        for jj in range(JG):
            for k in range(8):
                emit_transpose(zb_cur[jj], jj, k)
        for jg in range(n_jg):
            pending = []
            if jg + 1 < n_jg:
                zb_next = [load_and_cast((jg + 1) * JG + jj)
                           for jj in range(JG)]
                pending = [(zb_next[jj], (jg + 1) * JG + jj, k)
                           for jj in range(JG) for k in range(8)]
            pi = 0
            pos_stage = [None] * JG
            vel_stage = [None] * JG
            for m in range(NCH):
                ms = min(m, 1)
                for jj in range(JG):
                    j = jg * JG + jj
                    ksl = kt[m][jg][:, 128 * jj:128 * (jj + 1)]
                    bsl = slice(128 * j, 128 * (j + 1))

                    out_ps = ps_main.tile([128, 512], f32, tag="out")
                    nc.tensor.matmul(out_ps[:], zt[2 * m][:, bsl],
                                     wmain_sb[ms][0][:], start=True, stop=False)
                    nc.tensor.matmul(out_ps[:, 0:CW_N], ksl, cw_sb[ms][:],
                                     start=False, stop=False)
                    nc.tensor.matmul(out_ps[:], zt[2 * m + 1][:, bsl],
                                     wmain_sb[ms][1][:], start=False, stop=True)

                    for _ in range(2):
                        if pi < len(pending):
                            emit_transpose(*pending[pi])
                            pi += 1

                    if m < NCH - 1 and jj == JG - 1:
                        gsl = slice(128 * JG * jg, 128 * JG * (jg + 1))
                        cps = ps_c.tile([4, kw], f32, tag="carry")
                        nc.tensor.matmul(cps[:], bw_sb[ms][0][:],
                                         zt[2 * m][:, gsl],
                                         start=True, stop=False)
                        nc.tensor.matmul(cps[:], bw_sb[ms][1][:],
                                         zt[2 * m + 1][:, gsl],
                                         start=False, stop=False)
                        nc.tensor.matmul(cps[:], mw_sb[ms][:], kt[m][jg][:],
                                         start=False, stop=True)
                        nc.scalar.copy(kt[m + 1][jg][:], cps[:])

                    if m == 0:
                        pos_stage[jj] = stage.tile([128, 1024], f32,
                                                   name=f"pos_st_{j}",
                                                   tag=f"pos_st{jj}")
                        vel_stage[jj] = stage.tile([128, 1024], f32,
                                                   name=f"vel_st_{j}",
                                                   tag=f"vel_st{jj}")
                    csl = slice(256 * m, 256 * (m + 1))
                    nc.vector.tensor_copy(pos_stage[jj][:, csl],
                                          out_ps[:, 0:256])
                    nc.scalar.copy(vel_stage[jj][:, csl], out_ps[:, 256:512])
                    if m == NCH - 1:
                        nc.sync.dma_start(pos_out[bsl, :], pos_stage[jj][:])
                        nc.gpsimd.dma_start(vel_out[bsl, :], vel_stage[jj][:])

            while pi < len(pending):
                emit_transpose(*pending[pi])
                pi += 1

    nc.compile()
    return nc


# ----------------------------------------------------------------------------
# Host entry point
# ----------------------------------------------------------------------------

_CACHE = {}

# test-harness knobs (ignored in normal use)
PROFILE = False
LAST_RESULT = None


def _get_nc(n_bt):
    if n_bt not in _CACHE:
        _CACHE[n_bt] = build_nc(n_bt)
    return _CACHE[n_bt]


def kernel(pred_vel, dt, p0, q_pos, q_vel, r_vel):
    from concourse.bass_utils import run_bass_kernel_spmd

    z = np.ascontiguousarray(np.asarray(pred_vel, dtype=np.float32))
    p0 = np.ascontiguousarray(np.asarray(p0, dtype=np.float32))
    assert z.shape == (B_FULL, T, 2) and p0.shape == (B_FULL, 2)

    weights = build_weights(dt, q_pos, q_vel, r_vel)
    nc = _get_nc(B_CORE // 128)

    in_maps = []
    for i in range(N_CORES):
        sl = slice(i * B_CORE, (i + 1) * B_CORE)
        m = {"z_in": z[sl].reshape(B_CORE, 2 * T),
             "p0_in": p0[sl]}
        m.update(weights)
        in_maps.append(m)

    res = run_bass_kernel_spmd(nc, in_maps, core_ids=list(range(N_CORES)),
                               trace=PROFILE)
    global LAST_RESULT
    LAST_RESULT = res
    pos = np.concatenate([r["pos_out"].reshape(B_CORE, T, 2)
                          for r in res.results], axis=0)
    vel = np.concatenate([r["vel_out"].reshape(B_CORE, T, 2)
                          for r in res.results], axis=0)
    return pos, vel

